# revision 1
# baseline (speedup 1.0000x reference)
"""Expert-parallel MoE MLP (8 experts -> 8 NeuronCores) Bass kernel for TRN2.

Problem: y[t] = W2[e] @ gelu(W1[e] @ x[t] + b1[e]) + b2[e], tokens contiguous
per expert, 2048 tokens/expert, d_in=d_out=1024, d_hid=4096.

Sharding: expert-parallel. Core e gets expert e's weights and its 2048 tokens.
No cross-core communication needed (counts are equal and tokens are already
sorted by expert); host does the shard/unshard.

Per-core compute layout (everything partition-major, h kept as [hid, tok]):
  GEMM1: h[hid, tok]  = w1T[k,:].T @ xT[k, tok]   (accum over k = d_in tiles)
  GELU : h = gelu(h + b1)  via ScalarE with fused per-partition bias
  GEMM2: y[dout, tok] = w2T[k,:].T @ h[k, tok]    (accum over k = d_hid tiles)
  BIAS : y += b2 via ScalarE Identity with fused bias

The main loop runs at the PE streaming floor (~217 ns per 512-col matmul), so
the schedule optimizes the two ends:
  - prologue: DMA triggers run on BOTH HWDGE rings in parallel (Sync: w1/w2,
    Scalar: biases + x blocks), critical tiles first, one trigger per tile
    (each dma_start already stripes over all 16 SDMA engines); PE warmup sized
    to flip the HAM clock gate right as the first real tiles land.
  - tail: outputs are fp16 (host casts back), each output DMA is triggered on
    the Scalar engine directly after its bias-ACT, and the last output tile is
    computed as two 256-col PSUM groups so the final chunk drains early.
"""
import sys

sys.path.insert(0, "/opt/trn_rl_repo")

import numpy as np

import concourse.bass as bass  # noqa: F401
import concourse.tile as tile
from concourse import bacc, mybir
from concourse.bass_utils import run_bass_kernel_spmd

E = 8
T_PER_E = 2048
D_IN = 1024
D_HID = 4096
D_OUT = 1024

TOK_BLK = 512          # tokens per block (= PSUM bank free size in fp32)
N_TOK_BLK = T_PER_E // TOK_BLK
K1 = D_IN // 128       # k tiles for GEMM1
M1 = D_HID // 128      # output row tiles for GEMM1
K2 = D_HID // 128      # k tiles for GEMM2
M2 = D_OUT // 128      # output row tiles for GEMM2

CDT = mybir.dt.float16   # compute dtype on device (weights + activations)
NP_CDT = np.float16

N_WARM = 18              # 256-col warmup matmuls (~3.8 us cold) to flip HAM

_compiled = None


def _build():
    nc = bacc.Bacc("TRN2", target_bir_lowering=False, debug=False)

    # Host-permuted layouts (see _make_in_maps):
    #   xL [128, t*4096 + k*512 + c]   = x[t*512+c, k*128+p]
    #   w1L[128, m*1024 + k*128 + mc]  = w1[m*128+mc, k*128+p]
    #   w2L[128, d*4096 + k*128 + dc]  = w2[d*128+dc, k*128+p]
    #   bb [128, 0:32]=b1, [128, 32:40]=b2 (partition-transposed)
    xL = nc.dram_tensor("xL", [128, N_TOK_BLK * K1 * TOK_BLK], CDT, kind="ExternalInput").ap()
    w1L = nc.dram_tensor("w1L", [128, M1 * K1 * 128], CDT, kind="ExternalInput").ap()
    w2L = nc.dram_tensor("w2L", [128, M2 * K2 * 128], CDT, kind="ExternalInput").ap()
    bb = nc.dram_tensor("bb", [128, M1 + M2], mybir.dt.float32, kind="ExternalInput").ap()
    yT = nc.dram_tensor("yT", [D_OUT, T_PER_E], CDT, kind="ExternalOutput").ap()

    XBLK = K1 * TOK_BLK  # 4096 cols per token block in xL

    with tile.TileContext(nc) as tc:
        with tc.tile_pool(name="wpool", bufs=1) as wpool, \
             tc.tile_pool(name="xpool", bufs=1) as xpool, \
             tc.tile_pool(name="hpool", bufs=1) as hpool, \
             tc.tile_pool(name="opool", bufs=4) as opool, \
             tc.tile_pool(name="ps1", bufs=3, space="PSUM") as ps1, \
             tc.tile_pool(name="ps2", bufs=4, space="PSUM") as ps2:

            # --- PE warmup: keep the HAM activity window busy from kernel
            # entry until the first real tiles land (~3 us of cold matmuls),
            # so the real matmuls run at 2.4 GHz from the start ---
            scr = wpool.tile([128, 256], CDT, name="scr")
            nc.vector.memset(scr[:], 0.0)
            for i in range(N_WARM):
                wps = ps1.tile([128, 256], mybir.dt.float32, tag="ps1", name=f"warm{i}")
                nc.tensor.matmul(wps[:], scr[:, :128], scr[:], start=True, stop=True)

            # --- Prologue triggers, split across BOTH HWDGE rings so the
            # first tiles race in concurrently (one ring sustains ~205 GB/s;
            # two reach ~350-430).  Only what GEMM1(t0) needs up front goes
            # now: x(t0) striped over both rings in k-tile pieces, w1 m-tiles
            # in consumption order on Sync.  x(t1..t3) is deferred into the
            # main loop so it cannot starve the critical w1 stream. ---
            w1_sb = wpool.tile([128, M1 * K1 * 128], CDT, name="w1_sb")
            w2_sb = wpool.tile([128, M2 * K2 * 128], CDT, name="w2_sb")
            bb_sb = wpool.tile([128, M1 + M2], mybir.dt.float32, name="bb_sb")
            mw = K1 * 128
            dw = K2 * 128

            x_blocks = []
            for t in range(N_TOK_BLK):
                x_sb = xpool.tile([128, XBLK], CDT, tag=f"x{t}", name=f"x_sb{t}")
                x_blocks.append(x_sb)
            x0 = x_blocks[0]

            # Sync ring: ONLY the w1 stream (m0 in halves for a fast start,
            # then m1..m31), with w2 queued BEHIND it on the same ring so w2
            # cannot compete for SDMA bandwidth while GEMM1(t0) consumes w1
            # at ~150 GB/s.  Scalar ring: x(t0) (needed concurrently, k0
            # first and alone for the earliest possible first matmul) and the
            # biases; it goes quiet once x(t0) is in, leaving the ring to w1.
            nc.sync.dma_start(w1_sb[:, 0:mw // 2], w1L[:, 0:mw // 2])
            nc.sync.dma_start(w1_sb[:, mw // 2:mw], w1L[:, mw // 2:mw])
            nc.scalar.dma_start(x0[:, :TOK_BLK], xL[:, :TOK_BLK])
            nc.scalar.dma_start(bb_sb[:], bb[:, :])
            nc.scalar.dma_start(x0[:, TOK_BLK:2 * TOK_BLK],
                                xL[:, TOK_BLK:2 * TOK_BLK])
            nc.scalar.dma_start(x0[:, 2 * TOK_BLK:4 * TOK_BLK],
                                xL[:, 2 * TOK_BLK:4 * TOK_BLK])
            nc.scalar.dma_start(x0[:, 4 * TOK_BLK:6 * TOK_BLK],
                                xL[:, 4 * TOK_BLK:6 * TOK_BLK])
            nc.scalar.dma_start(x0[:, 6 * TOK_BLK:8 * TOK_BLK],
                                xL[:, 6 * TOK_BLK:8 * TOK_BLK])

            for m in range(1, M1):
                nc.sync.dma_start(w1_sb[:, m * mw:(m + 1) * mw],
                                  w1L[:, m * mw:(m + 1) * mw])
            for d in range(M2):
                nc.sync.dma_start(w2_sb[:, d * dw:(d + 1) * dw],
                                  w2L[:, d * dw:(d + 1) * dw])

            for t in range(N_TOK_BLK):
                x_sb = x_blocks[t]

                # --- GEMM1 + gelu: h[m] tiles ---
                h_tiles = []
                for m in range(M1):
                    psum = ps1.tile([128, TOK_BLK], mybir.dt.float32,
                                    tag="ps1", name=f"ps1_{t}_{m}")
                    for k in range(K1):
                        nc.tensor.matmul(
                            psum[:],
                            w1_sb[:, m * mw + k * 128: m * mw + (k + 1) * 128],
                            x_sb[:, k * TOK_BLK:(k + 1) * TOK_BLK],
                            start=(k == 0), stop=(k == K1 - 1),
                        )
                    h_sb = hpool.tile([128, TOK_BLK], CDT, tag=f"h{m}",
                                      name=f"h_sb{t}_{m}")
                    nc.scalar.activation(h_sb[:], psum[:],
                                         mybir.ActivationFunctionType.Gelu,
                                         bias=bb_sb[:, m:m + 1], scale=1.0)
                    h_tiles.append(h_sb)
                    # prefetch next token block once this block's weight
                    # stream is done competing for HBM (~late m-loop)
                    if m == 24 and t + 1 < N_TOK_BLK:
                        xn = x_blocks[t + 1]
                        nc.scalar.dma_start(
                            xn[:], xL[:, (t + 1) * XBLK:(t + 2) * XBLK])

                # --- GEMM2 + bias: y[d] tiles; output DMA is triggered from
                # the Scalar engine right after the ACT that produced it ---
                for d in range(M2):
                    last = (t == N_TOK_BLK - 1 and d == M2 - 1)
                    # the very last tile runs as shrinking PSUM groups so the
                    # final output chunk (128 cols) drains early
                    chunks = (((0, 256), (256, 384), (384, 512)) if last
                              else ((0, TOK_BLK),))
                    for (c0, c1) in chunks:
                        cw = c1 - c0
                        psum = ps2.tile([128, cw], mybir.dt.float32,
                                        tag="ps2", name=f"ps2_{t}_{d}_{c0}")
                        for k in range(K2):
                            nc.tensor.matmul(
                                psum[:],
                                w2_sb[:, d * dw + k * 128: d * dw + (k + 1) * 128],
                                h_tiles[k][:, c0:c1],
                                start=(k == 0), stop=(k == K2 - 1),
                            )
                        o_sb = opool.tile([128, cw], CDT,
                                          tag="o", name=f"o_sb{t}_{d}_{c0}")
                        nc.scalar.activation(o_sb[:], psum[:],
                                             mybir.ActivationFunctionType.Identity,
                                             bias=bb_sb[:, M1 + d:M1 + d + 1],
                                             scale=1.0)
                        nc.scalar.dma_start(
                            yT[d * 128:(d + 1) * 128,
                               t * TOK_BLK + c0:t * TOK_BLK + c1],
                            o_sb[:])

    nc.compile()
    return nc


def _get_compiled():
    global _compiled
    if _compiled is None:
        _compiled = _build()
    return _compiled


def _make_in_maps(x, w1, b1, w2, b2):
    in_maps = []
    for e in range(E):
        xe = x[e * T_PER_E:(e + 1) * T_PER_E]            # [2048, 1024]
        xl = xe.reshape(N_TOK_BLK, TOK_BLK, K1, 128)     # t, c, k, p
        xl = xl.transpose(3, 0, 2, 1).reshape(128, -1)   # p, (t k c)
        w1e = w1[e].reshape(M1, 128, K1, 128)            # m, mc, k, p
        w1l = w1e.transpose(3, 0, 2, 1).reshape(128, -1)  # p, (m k mc)
        w2e = w2[e].reshape(M2, 128, K2, 128)            # d, dc, k, p
        w2l = w2e.transpose(3, 0, 2, 1).reshape(128, -1)  # p, (d k dc)
        bbe = np.concatenate([b1[e].reshape(M1, 128).T,
                              b2[e].reshape(M2, 128).T], axis=1)
        in_maps.append({
            "xL": np.ascontiguousarray(xl).astype(NP_CDT),
            "w1L": np.ascontiguousarray(w1l).astype(NP_CDT),
            "w2L": np.ascontiguousarray(w2l).astype(NP_CDT),
            "bb": np.ascontiguousarray(bbe).astype(np.float32),
        })
    return in_maps


def run(x, cnt, w1, b1, w2, b2, trace=False):
    nc = _get_compiled()
    in_maps = _make_in_maps(x, w1, b1, w2, b2)
    res = run_bass_kernel_spmd(nc, in_maps, core_ids=list(range(E)), trace=trace)
    outs = [res.results[e]["yT"].T for e in range(E)]
    y = np.concatenate(outs, axis=0).astype(np.float32)
    return y, res


def kernel(x, cnt, w1, b1, w2, b2):
    y, _ = run(x, cnt, w1, b1, w2, b2, trace=False)
    return y

